# revision 10
# baseline (speedup 1.0000x reference)
"""Causal single-head attention on 8 Trainium2 NeuronCores, x-pair-exchange.

Problem: x [4, 2048, 1024], w_q/w_k/w_v [1024, 1024] (nn.Linear convention,
y = x @ W.T). Computes q,k,v projections, causal softmax(q k^T / sqrt(D)) @ v.

Math: scores S[i,t] = q_i . k_t = x_i^T (W_q^T W_k) x_t. The host
precomputes M = W_q^T W_k once (one 1024^3 sgemm - pure weight
preprocessing), the device computes z = M^T x_q per query and contracts
S^T[t,i] = x_t . z_i: the K projection disappears (folded into M) and the
pair-exchange ships RAW x^T instead of K^T, so the x AllGather needs no
projection to complete first.

Sharding: 2 cores per batch element. Core parity p owns token half
H_p = [p*1024,(p+1)*1024). Exchange (replica groups [[0,1],[2,3],[4,5],
[6,7]], executions serialize in trigger order): one 2MB x^T AG (bounced
from SBUF once the x tile lands, ~25us), then V in two 1MB AGs (token
quarters, V-b trigger fenced on the x-AG output so no more than two
pair-AGs are ever pending - 3+ concurrent corrupt the odd member,
measured). A framework start barrier floors the first AG at ~45us.
Queries: parity-interleaved 128-tiles (slot k has a kv window of 256k
tokens), host-gathered.

Attention computes S^T (token-chunk-major) so the softmax exp output IS
the P^T layout the AV matmul needs - no transposes. One token chunk c
serves every slot k >= floor(c/2)+1, whose query columns are contiguous in
z^T, so each chunk is 1-2 wide matmuls per e-chunk instead of one per
slot. exp reads straight from PSUM (no max subtraction - scores/sqrt(D)
are O(1)); per-slot row sums are accumulating ones-matmuls (~28ns each);
the causal mask (chunk c is the boundary of exactly its first slot column
block) is one extra accumulation matmul of identity @ maskT. AV runs as
two passes - first the chunks carried by the V-a AllGather for every
slot, then the V-b chunks, merged and scaled on the DVE - so the last AG
can arrive late without stalling the in-order PE queue.

DMA discipline (measured): a trigger costs ~650ns of HWDGE engine time;
descriptor generation is ~2.5-5ns per contiguous run, so every bulk
tensor is partition-major [128, W] (the host pre-tiles inputs into SBUF
layout); transfers are split into ~0.5-1MB pieces across both HWDGE
engines for parallelism; and each engine's DMA queue is ordered by
expected data-ready time, because a dependency-stalled DMA head-of-line
blocks everything behind it on that queue.
"""
import numpy as np
import ml_dtypes
from contextlib import ExitStack

import concourse.bass as bass
import concourse.tile as tile
import concourse.mybir as mybir
from concourse.bass_utils import run_bass_kernel_spmd
from concourse.masks import make_identity

F32 = mybir.dt.float32
BF16 = mybir.dt.bfloat16
AF = mybir.ActivationFunctionType
AX = mybir.AxisListType

B, S, E, D = 4, 2048, 1024, 1024
NCORES = 8
NSLOT = 8              # slots k=1..8, kv window = 256*k tokens
NQ = NSLOT * 128       # queries per core
HT = S // 2            # tokens owned per core (own half)
EC = E // 128          # e-chunks
NCH = S // 128         # token chunks
XW = EC * HT           # columns of one core's x^T tile (8192)
SCALE = 1.0 / 32.0     # 1/sqrt(D)
MASKVAL = -30000.0
GROUPS = [[0, 1], [2, 3], [4, 5], [6, 7]]

_prog = None


def _kmin(c):
    """First slot whose kv window includes token chunk c."""
    return c // 2 + 1


def _split_multi_waits(nc, max_waits=1):
    """The walrus build in this container has one sync-wait slot per
    instruction; hoist extra waits onto preceding same-engine NoOps."""
    n = 0
    for f in nc.m.functions:
        for b in f.blocks:
            insts = b.instructions
            out = []
            changed = False
            for ins in insts:
                si = ins.sync_info
                if si is not None and len(si.on_wait) > max_waits:
                    waits = list(si.on_wait)
                    for w in waits[:-max_waits]:
                        nop = mybir.InstNoOp(name=f"I-waitsplit-{n}")
                        n += 1
                        nop.engine = ins.engine
                        nop.sync_info = mybir.SyncInfo(on_wait=[w], on_update=[])
                        out.append(nop)
                    ins.sync_info = mybir.SyncInfo(
                        on_wait=waits[-max_waits:], on_update=list(si.on_update))
                    changed = True
                out.append(ins)
            if changed:
                b.instructions = out
    return nc


def _build(split=True):
    nc = bass.Bass(trn_type="TRN2", target_bir_lowering=False, debug=False)
    # all bulk inputs are host-pre-tiled to partition-major SBUF layout
    xo_in = nc.dram_tensor("xo", [128, XW], BF16, kind="ExternalInput").ap()
    xq_in = nc.dram_tensor("xq", [128, EC * NQ], BF16, kind="ExternalInput").ap()
    m_in = nc.dram_tensor("m", [128, EC * E], BF16, kind="ExternalInput").ap()
    wv_in = nc.dram_tensor("wv", [128, EC * D], BF16, kind="ExternalInput").ap()
    maskin = nc.dram_tensor("maskT", [256, 128], BF16, kind="ExternalInput").ap()
    onesin = nc.dram_tensor("ones", [128, 1], BF16, kind="ExternalInput").ap()
    out = nc.dram_tensor("out", [NQ, D], F32, kind="ExternalOutput").ap()

    bncX = nc.dram_tensor("bncX", [128, XW], BF16).ap()
    gathX = nc.dram_tensor("gathX", [2, 128, XW], BF16).ap()
    bncVa = nc.dram_tensor("bncVa", [128, 4 * D], BF16).ap()
    gathVa = nc.dram_tensor("gathVa", [2, 128, 4 * D], BF16).ap()
    bncVb = nc.dram_tensor("bncVb", [128, 4 * D + 16], BF16).ap()
    gathVb = nc.dram_tensor("gathVb", [2, 128, 4 * D + 16], BF16).ap()

    with tile.TileContext(nc) as tc, ExitStack() as ctx:
        # x^T bounce pieces straight from the input (DRAM->DRAM), split
        # across both HWDGE engines at t~0: the x AG triggers ~14us in
        for piece in range(2):
            lo = piece * (XW // 4)
            nc.sync.dma_start(bncX[:, lo:lo + XW // 4],
                              xo_in[:, lo:lo + XW // 4])
        for piece in range(2, 4):
            lo = piece * (XW // 4)
            nc.scalar.dma_start(bncX[:, lo:lo + XW // 4],
                                xo_in[:, lo:lo + XW // 4])
        nc.gpsimd.collective_compute(
            "AllGather", mybir.AluOpType.bypass, replica_groups=GROUPS,
            ins=[bncX.opt()], outs=[gathX.opt()])

        const = ctx.enter_context(tc.tile_pool(name="const", bufs=1))
        ident = const.tile([128, 128], BF16)
        make_identity(nc, ident[:])
        maskT = const.tile([128, 256], BF16)   # [:, 0:128]=rows 0:128, etc
        nc.scalar.dma_start(maskT[:, 0:128], maskin[0:128, :])
        nc.scalar.dma_start(maskT[:, 128:256], maskin[128:256, :])
        ones = const.tile([128, 1], BF16)
        nc.scalar.dma_start(ones[:], onesin[:])

        # z^T stays resident until the end of attention. col = e*NQ + q
        qtp = ctx.enter_context(tc.tile_pool(name="qtp", bufs=1))
        zts = qtp.tile([128, EC * NQ], BF16, name="zts")

        # ---- Phase 1: x -> AG; V_own -> AGs; z = M^T x_q ----
        with tc.tile_pool(name="wp", bufs=1) as wp, \
             tc.tile_pool(name="xp", bufs=1) as xp, \
             tc.tile_pool(name="st", bufs=1) as stp, \
             tc.tile_pool(name="ps1", bufs=4, space="PSUM") as pp:
            # xo col = e*HT + t ; wv/m cols = e*1024 + c
            wv = wp.tile([128, EC * D], BF16, name="wv")
            m = wp.tile([128, EC * E], BF16, name="m")
            xo = xp.tile([128, XW], BF16, name="xo")
            xq = xp.tile([128, EC * NQ], BF16, name="xq")

            # interleave xo/wv pieces (V runs first), then m and xq for z
            for piece in range(4):
                xw = XW // 4
                nc.sync.dma_start(xo[:, piece * xw:(piece + 1) * xw],
                                  xo_in[:, piece * xw:(piece + 1) * xw])
                ww = EC * D // 4
                nc.sync.dma_start(wv[:, piece * ww:(piece + 1) * ww],
                                  wv_in[:, piece * ww:(piece + 1) * ww])
            for piece in range(2):
                mw = EC * E // 2
                nc.sync.dma_start(m[:, piece * mw:(piece + 1) * mw],
                                  m_in[:, piece * mw:(piece + 1) * mw])
            for piece in range(2):
                qw = EC * NQ // 2
                nc.sync.dma_start(xq[:, piece * qw:(piece + 1) * qw],
                                  xq_in[:, piece * qw:(piece + 1) * qw])

            # V_own: stationary x chunks, moving wv. vown col = t*D + c
            vown = stp.tile([128, (HT // 128) * D], BF16, name="vown")
            for v in range(2):
                for tl in range(4):
                    t = v * 4 + tl
                    for h in range(2):
                        ps = pp.tile([128, 512], F32, name=f"pv{t}_{h}", tag="pp")
                        for e in range(EC):
                            nc.tensor.matmul(
                                ps[:],
                                xo[:, e * HT + t * 128:e * HT + (t + 1) * 128],
                                wv[:, e * D + h * 512:e * D + (h + 1) * 512],
                                start=(e == 0), stop=(e == EC - 1))
                        nc.vector.tensor_copy(
                            vown[:, t * D + h * 512:t * D + (h + 1) * 512],
                            ps[:])
                if v == 0:
                    nc.scalar.dma_start(bncVa[:], vown[:, 0:4 * D])
                    nc.gpsimd.collective_compute(
                        "AllGather", mybir.AluOpType.bypass,
                        replica_groups=GROUPS,
                        ins=[bncVa.opt()], outs=[gathVa.opt()])
                else:
                    nc.scalar.dma_start(bncVb[:, 0:4 * D],
                                        vown[:, 4 * D:8 * D])
                    # fence: V-b may only trigger once the x AG has fully
                    # delivered (keeps pending+running pair-AGs <= 2)
                    nc.scalar.dma_start(bncVb[0:1, 4 * D:4 * D + 16],
                                        gathX[1, 0:1, 0:16])
                    nc.gpsimd.collective_compute(
                        "AllGather", mybir.AluOpType.bypass,
                        replica_groups=GROUPS,
                        ins=[bncVb.opt()], outs=[gathVb.opt()])

            # z^T = M^T x_q: stationary M chunks, moving xq. col = e*NQ + q.
            for d in range(EC):
                for g in range(2):
                    ps = pp.tile([128, 512], F32, name=f"pq{d}_{g}", tag="pp")
                    for e in range(EC):
                        nc.tensor.matmul(
                            ps[:],
                            m[:, e * E + d * 128:e * E + (d + 1) * 128],
                            xq[:, e * NQ + g * 512:e * NQ + (g + 1) * 512],
                            start=(e == 0), stop=(e == EC - 1))
                    nc.vector.tensor_copy(
                        zts[:, d * NQ + g * 512:d * NQ + (g + 1) * 512], ps[:])

        # ---- Phase 2: load gathered x^T / V into SBUF ----
        # xts col = r*XW + e*HT + t_local ; vts col = t*D + c (global chunk)
        # pieces split across both HWDGE engines, each queue ordered by
        # expected ready time (head-of-line discipline)
        kvp = ctx.enter_context(tc.tile_pool(name="kvp", bufs=1))
        xts = kvp.tile([128, 2 * XW], BF16, name="xts")
        vts = kvp.tile([128, NCH * D], BF16, name="vts")
        for r in range(2):
            for piece in range(4):
                eng = nc.sync if piece % 2 == 0 else nc.scalar
                lo = piece * (XW // 4)
                eng.dma_start(xts[:, r * XW + lo:r * XW + lo + XW // 4],
                              gathX[r, :, lo:lo + XW // 4])
        for r in range(2):   # V-a quarters: global chunks 0-3 (r0), 8-11 (r1)
            eng = nc.sync if r == 0 else nc.scalar
            t0 = r * 8
            eng.dma_start(vts[:, t0 * D:(t0 + 4) * D], gathVa[r])
        for r in range(2):   # V-b quarters: global chunks 4-7 (r0), 12-15 (r1)
            eng = nc.sync if r == 0 else nc.scalar
            t0 = r * 8 + 4
            eng.dma_start(vts[:, t0 * D:(t0 + 4) * D], gathVb[r, :, 0:4 * D])

        def xtc(c, e):
            """xts col of (global token chunk c, e-chunk e)."""
            return (c // 8) * XW + e * HT + (c % 8) * 128

        # ---- Phase 3: chunk-major S^T scores + softmax (P^T straight) ----
        att = ctx.enter_context(tc.tile_pool(name="att", bufs=1))
        stats = ctx.enter_context(tc.tile_pool(name="stats", bufs=1))
        linv = stats.tile([128, NSLOT], F32, name="linv")
        # per-chunk P^T tiles: cols = slots kmin(c)..8, 128 each
        pT = {c: att.tile([128, 128 * (NSLOT + 1 - _kmin(c))], BF16,
                          name=f"pT{c}") for c in range(NCH)}
        osb = {k: att.tile([128, D], F32, name=f"osb{k}")
               for k in range(3, NSLOT + 1)}
        av_a = {k: [c for c in range(2 * k) if c % 8 < 4]
                for k in range(1, NSLOT + 1)}
        av_b = {k: [c for c in range(2 * k) if c % 8 >= 4]
                for k in range(1, NSLOT + 1)}

        with tc.tile_pool(name="ps3", bufs=1, space="PSUM") as pp3:
            ls = pp3.tile([128, 2], F32, name="ls", tag="lsp", bufs=1)

            def emit_lsum(k):
                for ci, c in enumerate(range(2 * k)):
                    j = k - _kmin(c)
                    nc.tensor.matmul(ls[:, 0:1],
                                     pT[c][:, j * 128:(j + 1) * 128],
                                     ones[:], start=(ci == 0),
                                     stop=(ci == 2 * k - 1))
                nc.vector.reciprocal(linv[:, k - 1:k], ls[:, 0:1])

            for c in range(NCH):
                km = _kmin(c)
                w = 128 * (NSLOT + 1 - km)
                npc = (w + 511) // 512
                sT = [pp3.tile([128, 512], F32, name=f"sT{c}_{i}", tag="sps",
                               bufs=4) for i in range(npc)]
                for i in range(npc):
                    pw = min(512, w - i * 512)
                    qoff = (km - 1) * 128 + i * 512
                    msk = (i == 0)
                    for e in range(EC):
                        nc.tensor.matmul(
                            sT[i][:, :pw],
                            xts[:, xtc(c, e):xtc(c, e) + 128],
                            zts[:, e * NQ + qoff:e * NQ + qoff + pw],
                            start=(e == 0), stop=(e == EC - 1 and not msk))
                    if msk:
                        # chunk c is the causal boundary of slot kmin(c),
                        # which owns this chunk's first 128 query columns
                        mo = 0 if c % 2 == 0 else 128
                        nc.tensor.matmul(sT[i][:, 0:128], ident[:],
                                         maskT[:, mo:mo + 128],
                                         start=False, stop=True,
                                         skip_group_check=True)
                    nc.scalar.activation(pT[c][:, i * 512:i * 512 + pw],
                                         sT[i][:, :pw], AF.Exp, scale=SCALE)
                # slot k's last chunk is 2k-1; emit its row-sum matmuls one
                # chunk later so the PE never waits on the exp it just fed
                if c >= 2 and c % 2 == 0:
                    emit_lsum(c // 2)
            emit_lsum(NSLOT)

            # ---- Phase 4a: AV over the V-a chunks for every slot ----
            for k in range(1, NSLOT + 1):
                ca = av_a[k]
                o_ps = [pp3.tile([128, 512], F32, name=f"oa{k}_{h}", tag="ops",
                                 bufs=3) for h in range(2)]
                for ci, c in enumerate(ca):
                    j = k - _kmin(c)
                    for h in range(2):
                        nc.tensor.matmul(o_ps[h][:],
                                         pT[c][:, j * 128:(j + 1) * 128],
                                         vts[:, c * D + h * 512:c * D + (h + 1) * 512],
                                         start=(ci == 0), stop=(ci == len(ca) - 1))
                if not av_b[k]:
                    # k=1,2: fully V-a resident; normalize and write out
                    o_fin = att.tile([128, D], F32, name=f"ofa{k}", tag="ofin",
                                     bufs=2)
                    for h in range(2):
                        nc.scalar.activation(o_fin[:, h * 512:(h + 1) * 512],
                                             o_ps[h][:], AF.Copy,
                                             scale=linv[:, k - 1:k])
                    nc.sync.dma_start(out[(k - 1) * 128:k * 128, :], o_fin[:])
                else:
                    # stash normalized partial on the DVE; V-b added in 4b
                    for h in range(2):
                        nc.vector.tensor_scalar_mul(
                            osb[k][:, h * 512:(h + 1) * 512], o_ps[h][:],
                            linv[:, k - 1:k])

            # ---- Phase 4b: AV over the V-b chunks, scale+add on DVE ----
            for k in range(3, NSLOT + 1):
                cb = av_b[k]
                o_ps = [pp3.tile([128, 512], F32, name=f"ob{k}_{h}", tag="ops",
                                 bufs=3) for h in range(2)]
                for ci, c in enumerate(cb):
                    j = k - _kmin(c)
                    for h in range(2):
                        nc.tensor.matmul(o_ps[h][:],
                                         pT[c][:, j * 128:(j + 1) * 128],
                                         vts[:, c * D + h * 512:c * D + (h + 1) * 512],
                                         start=(ci == 0), stop=(ci == len(cb) - 1))
                o_fin = att.tile([128, D], F32, name=f"ofb{k}", tag="ofin",
                                 bufs=2)
                o_sc = att.tile([128, D], F32, name=f"osc{k}", tag="osc", bufs=2)
                for h in range(2):
                    hs = slice(h * 512, (h + 1) * 512)
                    nc.vector.tensor_scalar_mul(o_sc[:, hs], o_ps[h][:],
                                                linv[:, k - 1:k])
                    nc.vector.tensor_add(o_fin[:, hs], o_sc[:, hs], osb[k][:, hs])
                nc.sync.dma_start(out[(k - 1) * 128:k * 128, :], o_fin[:])
    if split:
        _split_multi_waits(nc)
    return nc


def _masks():
    """Transposed boundary masks [256 window rows, 128 query cols], bf16."""
    j = np.arange(256)[:, None]
    i = np.arange(128)[None, :]
    bf = ml_dtypes.bfloat16
    maskT0 = np.where(j <= i, 0.0, MASKVAL).astype(bf)          # parity 0
    maskT1 = np.where(j <= 128 + i, 0.0, MASKVAL).astype(bf)    # parity 1
    return maskT0, maskT1


def _ptile(a):
    """[E, W] -> partition-major [128, EC*W]: out[p, e*W+c] = a[e*128+p, c]."""
    Erows, W = a.shape
    ec = Erows // 128
    return np.ascontiguousarray(
        a.reshape(ec, 128, W).transpose(1, 0, 2).reshape(128, ec * W))


def _in_maps(x, w_q, w_k, w_v):
    bf = ml_dtypes.bfloat16
    x = np.asarray(x, np.float32)
    # fold the Q and K projections into one host-side matrix:
    # S[i,t] = x_i^T (W_q^T W_k) x_t
    m = (np.asarray(w_q, np.float32).T @ np.asarray(w_k, np.float32))
    m_t = _ptile(m.astype(bf))
    wv_t = _ptile(np.ascontiguousarray(np.asarray(w_v, np.float32).T).astype(bf))
    maskT0, maskT1 = _masks()
    ones = np.ones((128, 1), dtype=bf)

    in_maps = []
    for c in range(NCORES):
        b, p = divmod(c, 2)
        xb = x[b]                                    # [S, E]
        xoT = np.ascontiguousarray(xb[p * HT:(p + 1) * HT, :].T).astype(bf)
        qrows = np.concatenate(
            [xb[128 * (2 * (k - 1) + p):128 * (2 * (k - 1) + p) + 128, :]
             for k in range(1, NSLOT + 1)], axis=0)  # [NQ, E]
        xq_t = _ptile(np.ascontiguousarray(qrows.T).astype(bf))
        in_maps.append({
            "xo": _ptile(xoT), "xq": xq_t,
            "m": m_t, "wv": wv_t,
            "maskT": maskT0 if p == 0 else maskT1,
            "ones": ones,
        })
    return in_maps


def _scatter(per_core_out):
    out = np.empty((B, S, D), dtype=np.float32)
    for c in range(NCORES):
        b, p = divmod(c, 2)
        oc = per_core_out[c]                         # [NQ, D]
        for k in range(1, NSLOT + 1):
            g = 2 * (k - 1) + p
            out[b, 128 * g:128 * (g + 1), :] = oc[128 * (k - 1):128 * k, :]
    return out


def kernel(x, w_q, w_k, w_v):
    global _prog
    if _prog is None:
        _prog = _build()
    in_maps = _in_maps(x, w_q, w_k, w_v)
    res = run_bass_kernel_spmd(_prog, in_maps, list(range(NCORES)))
    return _scatter([res.results[c]["out"] for c in range(NCORES)])
